# revision 24
# baseline (speedup 1.0000x reference)
"""Causal attention (RoPE, 16 heads, L=2048, H=2048) on 8 trn2 NeuronCores.

Sharding: tensor-parallel over heads. Core i handles heads 2i, 2i+1
(d=128 each): column-parallel q/k/v projections, row-parallel o_proj,
host-side sum of the 8 partial outputs.

Per-core device kernel (all matmuls fp32r = full-rate PE):
  - Q^T, K^T computed in [d, L] layout (weight-stationary matmuls, N=512),
    RoPE applied during PSUM->SBUF eviction on DVE.
  - V computed via PE transposes into natural [L, d] layout.
  - Weights streamed in kc-paired chunks on both hwdge queues, interleaved
    with x tiles (the lc=0 pass is DMA-bandwidth-bound).
  - Attention per (head, q-chunk of 512): S^T = K Q^T on PE; causal mask
    applied as an additive -1e30 bias via an identity matmul on the PE
    itself (no cross-engine dependency in the S->exp chain); diagonal
    blocks narrowed to the causal frontier; exp on ACT; softmax
    denominators via an all-ones matmul; unnormalized O^T accumulated
    over k-tiles; normalized by DVE reciprocal+mul into [d, L] layout.
  - o_proj: out_partial[q, H] from lhsT=O^T chunks, rhs=Wo slice.
"""
import numpy as np

L = 2048
H = 2048
NH = 16
D = 128          # head dim
NCORES = 8
HPC = NH // NCORES   # heads per core = 2
ROPE_BASE = 10000.0

_CACHE = {}


def _rope_tables():
    inv_freq = 1.0 / (ROPE_BASE ** (np.arange(0, D, 2, dtype=np.float32) / D))
    t = np.arange(L, dtype=np.float32)
    freqs = np.outer(t, inv_freq).astype(np.float32)          # [L, D/2]
    emb = np.concatenate([freqs, freqs], axis=-1)             # [L, D]
    cos = np.cos(emb).astype(np.float32)                      # [L, D]
    sin = np.sin(emb).astype(np.float32)
    cosT = np.ascontiguousarray(cos.T)                        # [D, L]
    sinT = np.ascontiguousarray(sin.T)
    sinTs = sinT.copy()
    sinTs[: D // 2] = -sinT[: D // 2]                         # sign-folded
    # partition-swapped so DVE operand base partitions match:
    # sinsw[p] = sinTs[(p+64) % 128]
    sinsw = np.concatenate([sinTs[D // 2:], sinTs[: D // 2]], axis=0)
    return cosT, np.ascontiguousarray(sinsw)


def _mask_bias():
    # maskb[j][k, q] = 0 where q - k - 128*j >= 0 else -1e30
    k = np.arange(128)[None, :, None]
    q = np.arange(512)[None, None, :]
    j = np.arange(4)[:, None, None]
    keep = (q - k - 128 * j) >= 0
    return np.where(keep, np.float32(0.0), np.float32(-1e30))


def _build_nc():
    import concourse.bacc as bacc
    import concourse.mybir as mybir
    from concourse import tile
    from contextlib import ExitStack

    f32 = mybir.dt.float32
    f32r = mybir.dt.float32r
    AF = mybir.ActivationFunctionType
    OP = mybir.AluOpType

    nc = bacc.Bacc("TRN2", target_bir_lowering=False, debug=False)

    xT_d = nc.dram_tensor("xT", (H, L), f32r, kind="ExternalInput")
    wq_d = nc.dram_tensor("wqT", (H // 128, 128, HPC * D), f32r, kind="ExternalInput")
    wk_d = nc.dram_tensor("wkT", (H // 128, 128, HPC * D), f32r, kind="ExternalInput")
    wv_d = nc.dram_tensor("wvT", (H // 128, 128, HPC * D), f32r, kind="ExternalInput")
    wo_d = nc.dram_tensor("woP", (HPC * D, H), f32r, kind="ExternalInput")
    cos_d = nc.dram_tensor("cosT", (D, L), f32, kind="ExternalInput")
    sin_d = nc.dram_tensor("sinTs", (D, L), f32, kind="ExternalInput")
    mb_d = nc.dram_tensor("maskb", (4, 128, 512), f32r, kind="ExternalInput")
    id_d = nc.dram_tensor("ident", (128, 128), f32r, kind="ExternalInput")
    out_d = nc.dram_tensor("out", (L, H), f32, kind="ExternalOutput")

    KC = H // 128        # 16 contraction chunks
    LCN = 4              # L chunks of 512 in projections
    QCN = 4              # q chunks of 512 in attention

    with tile.TileContext(nc) as tc, ExitStack() as top:
        per = top.enter_context(tc.tile_pool(name="per", bufs=1))

        wo_sb = per.tile([128, HPC, H], f32r)
        cos_sb = per.tile([128, L], f32)
        sin_sb = per.tile([128, L], f32)
        qt_sb = [[per.tile([128, 512], f32r, name=f"qt{h}_{c}")
                  for c in range(LCN)] for h in range(HPC)]
        kt_sb = [[per.tile([128, 512], f32r, name=f"kt{h}_{c}")
                  for c in range(LCN)] for h in range(HPC)]
        v_sb = [[per.tile([128, 4, D], f32r, name=f"v{h}_{c}")
                 for c in range(LCN)] for h in range(HPC)]
        ot_sb = [[per.tile([128, 512], f32r, name=f"ot{h}_{c}")
                  for c in range(QCN)] for h in range(HPC)]
        ones_f = per.tile([128, 128], f32)
        ones_r = per.tile([128, 128], f32r)
        ident = per.tile([128, 128], f32r)

        nc.vector.memset(ones_f[:], 1.0)
        nc.vector.tensor_copy(ones_r[:], ones_f[:])
        nc.gpsimd.dma_start(ident[:], id_d[:])

        # ---------------- projections ----------------
        # Head-split passes: each (lc, h) group runs q/k/v for ONE head
        # (3 PSUM banks, 3MB of weights), so only half the weight bytes gate
        # the first matmuls and each group's eviction overlaps the next
        # group's ~12us of PE work (per-h bank tags alternate).
        with ExitStack() as proj:
            wpool = proj.enter_context(tc.tile_pool(name="w", bufs=1))
            xpool = proj.enter_context(tc.tile_pool(name="x", bufs=16))
            tpool = proj.enter_context(tc.tile_pool(name="ropetmp", bufs=2))
            spool = proj.enter_context(tc.tile_pool(name="stg", bufs=1))
            vtpool = proj.enter_context(tc.tile_pool(name="vt", bufs=2))
            qpps = proj.enter_context(tc.tile_pool(name="qpps", bufs=1, space="PSUM"))
            pps = proj.enter_context(tc.tile_pool(name="pps", bufs=1, space="PSUM"))
            tpps = proj.enter_context(tc.tile_pool(name="tpps", bufs=2, space="PSUM"))

            wq_sb = wpool.tile([128, KC, HPC * D], f32r)
            wk_sb = wpool.tile([128, KC, HPC * D], f32r)
            wv_sb = wpool.tile([128, KC, HPC * D], f32r)
            wqr, wkr, wvr = wq_d, wk_d, wv_d

            # per-head weight chunks: first kc alone (gates the first
            # matmul), then pairs; each type on its own queue
            wchunks = [(0, 1), (1, 3), (3, 5), (5, 7), (7, 9), (9, 11),
                       (11, 13), (13, 16)]

            def wdma(eng, dst, src, ks, h):
                hs = slice(h * D, (h + 1) * D)
                eng.dma_start(dst[:, ks, hs],
                              src[ks, :, hs].rearrange("c p n -> p c n"))

            def rope(src, dst, cs):
                # dst = src*cos + swap(src)*sin_signed  (DVE, f32r out)
                t1 = tpool.tile([128, 512], f32, tag="t1", name="t1")
                t2 = tpool.tile([128, 512], f32, tag="t2", name="t2")
                nc.vector.tensor_tensor(
                    t1[0:64, :], src[64:128, :], sin_sb[64:128, cs], OP.mult)
                nc.vector.tensor_tensor(
                    t1[64:128, :], src[0:64, :], sin_sb[0:64, cs], OP.mult)
                nc.vector.tensor_tensor(t2[:], src[:], cos_sb[:, cs], OP.mult)
                nc.vector.tensor_tensor(dst[:], t1[:], t2[:], OP.add)

            xts = []
            for lc in range(LCN):
                cs = slice(lc * 512, (lc + 1) * 512)
                new_xts = []
                for h in range(HPC):
                    qps = qpps.tile([128, 512], f32, tag=f"qps{h}", name=f"qps{h}")
                    kps = pps.tile([128, 512], f32, tag=f"kps{h}", name=f"kps{h}")
                    vps = pps.tile([128, 512], f32, tag=f"vps{h}", name=f"vps{h}")
                    for kc in range(KC):
                        if h == 0:
                            xt = xpool.tile([128, 512], f32r, tag="xt")
                            nc.sync.dma_start(
                                xt[:],
                                xT_d[kc * 128:(kc + 1) * 128, lc * 512:(lc + 1) * 512])
                            new_xts.append(xt)
                        xt = new_xts[kc]
                        if lc == 0:
                            for lo, hi in wchunks:
                                if lo == kc:
                                    ks = slice(lo, hi)
                                    wdma(nc.scalar, wq_sb, wqr, ks, h)
                                    wdma(nc.sync, wk_sb, wkr, ks, h)
                                    wdma(nc.gpsimd, wv_sb, wvr, ks, h)
                        if kc == 8 and (lc, h) in ((0, 0), (0, 1), (1, 0), (1, 1)):
                            # cos/sin chunk c only needed by RoPE of lc=c
                            ci = 2 * lc + h
                            ccs = slice(ci * 512, (ci + 1) * 512)
                            nc.scalar.dma_start(cos_sb[:, ccs], cos_d[:, ccs])
                            nc.sync.dma_start(sin_sb[:, ccs], sin_d[:, ccs])
                        st, sp = (kc == 0), (kc == KC - 1)
                        nc.tensor.matmul(qps[:], wq_sb[:, kc, h * D:(h + 1) * D],
                                         xt[:], start=st, stop=sp)
                        nc.tensor.matmul(kps[:], wk_sb[:, kc, h * D:(h + 1) * D],
                                         xt[:], start=st, stop=sp)
                        nc.tensor.matmul(vps[:], wv_sb[:, kc, h * D:(h + 1) * D],
                                         xt[:], start=st, stop=sp)
                    # evictions: PSUM -> SBUF copies on ACT (banks free fast),
                    # RoPE on DVE; all overlap the next group's matmuls
                    stg = spool.tile([128, 512], f32, tag=f"qs{h}", name=f"qs{h}")
                    nc.scalar.copy(stg[:], qps[:])
                    rope(stg, qt_sb[h][lc], cs)
                    stg = spool.tile([128, 512], f32, tag=f"ks{h}", name=f"ks{h}")
                    nc.scalar.copy(stg[:], kps[:])
                    rope(stg, kt_sb[h][lc], cs)
                    vt = vtpool.tile([128, 512], f32r, tag="vt")
                    nc.scalar.copy(vt[:], vps[:])
                    for lt in range(4):
                        tp = tpps.tile([128, 128], f32r, tag="tp", name="tp")
                        nc.tensor.transpose(tp[:], vt[:, lt * 128:(lt + 1) * 128],
                                            ident[:])
                        nc.scalar.copy(v_sb[h][lc][:, lt, :], tp[:])
                xts = new_xts

        # -------- attention (qc-major) interleaved with o_proj --------
        with ExitStack() as att:
            ppool = att.enter_context(tc.tile_pool(name="pt", bufs=8))
            rpool = att.enter_context(tc.tile_pool(name="recip", bufs=2))
            obuf = att.enter_context(tc.tile_pool(name="ob", bufs=10))
            mpool = att.enter_context(tc.tile_pool(name="mb", bufs=1))
            sps_p = att.enter_context(tc.tile_pool(name="sps", bufs=2, space="PSUM"))
            acc_p = att.enter_context(tc.tile_pool(name="acc", bufs=2, space="PSUM"))
            ops_p = att.enter_context(tc.tile_pool(name="ops", bufs=2, space="PSUM"))

            maskb_sb = mpool.tile([128, 4, 512], f32r)
            nc.gpsimd.dma_start(maskb_sb[:], mb_d.rearrange("j p n -> p j n"))

            for qc in range(QCN):
                n_kt = 4 * qc + 4
                # interleave the two heads' chains so the PE always has an
                # independent stream while one head's exp is in flight
                sums = [acc_p.tile([128, 512], f32, tag="sums", name=f"sums{h}")
                        for h in range(HPC)]
                ops = [acc_p.tile([128, 512], f32, tag="ops", name=f"ops{h}")
                       for h in range(HPC)]
                for kt in range(n_kt):
                    st, sp = (kt == 0), (kt == n_kt - 1)
                    diag = kt >= 4 * qc
                    # diagonal blocks: columns below the causal frontier are
                    # fully masked -- skip them. j=3 keeps width 256 (a
                    # 128-wide fp32r matmul drops to 1/4 rate).
                    off = min(128 * (kt - 4 * qc), 256) if diag else 0
                    qs = slice(off, 512)
                    pts = []
                    for h in range(HPC):
                        s_ps = sps_p.tile([128, 512], f32, tag="s")
                        nc.tensor.matmul(
                            s_ps[:, qs], kt_sb[h][kt // 4][:, (kt % 4) * 128:(kt % 4 + 1) * 128],
                            qt_sb[h][qc][:, qs], start=True, stop=not diag)
                        if diag:
                            # causal mask: additive -1e30 bias injected by an
                            # identity matmul on the PE itself; only the
                            # 256-wide stripe around the frontier is nonzero
                            j = kt - 4 * qc
                            bs = slice(off, off + 256)
                            nc.tensor.matmul(s_ps[:, bs], ident[:],
                                             maskb_sb[:, j, bs],
                                             start=False, stop=True)
                        pt = ppool.tile([128, 512], f32r, tag="pt")
                        nc.scalar.activation(pt[:, qs], s_ps[:, qs], AF.Exp)
                        pts.append(pt)
                    # both heads' denominator matmuls back-to-back: they
                    # share the all-ones stationary operand
                    for h in range(HPC):
                        nc.tensor.matmul(sums[h][:, qs], ones_r[:], pts[h][:, qs],
                                         start=st, stop=sp)
                    for h in range(HPC):
                        nc.tensor.matmul(ops[h][:, qs], v_sb[h][kt // 4][:, kt % 4, :],
                                         pts[h][:, qs], start=st, stop=sp)
                if qc == 0:
                    # wo arrives during the attention phase, well before the
                    # first o_proj; issued on sync so the ACT sequencer (busy
                    # with the exp chain) never stalls on DMA setup
                    nc.sync.dma_start(wo_sb[:],
                                      wo_d.rearrange("(c p) n -> p c n", p=128))
                for h in range(HPC):
                    recip = rpool.tile([128, 512], f32, tag="recip")
                    nc.vector.reciprocal_approx_fast(recip[:], sums[h][:])
                    nc.vector.tensor_tensor(ot_sb[h][qc][:], ops[h][:], recip[:],
                                            OP.mult)
                # o_proj for the q-tiles of this q-chunk (overlaps the next
                # q-chunk's attention); the last chunk runs after the att
                # pools close, with a deeper dedicated pool
                if qc == QCN - 1:
                    continue
                for qt4 in range(4):
                    for hcn in range(4):
                        po = ops_p.tile([128, 512], f32, tag="po")
                        for h in range(HPC):
                            nc.tensor.matmul(
                                po[:], ot_sb[h][qc][:, qt4 * 128:(qt4 + 1) * 128],
                                wo_sb[:, h, hcn * 512:(hcn + 1) * 512],
                                start=(h == 0), stop=(h == HPC - 1))
                        ob = obuf.tile([128, 512], f32, tag="ob")
                        nc.vector.tensor_copy(ob[:], po[:])
                        qt = qc * 4 + qt4
                        # out-DMAs on sync/swdge only: a scalar-queue issue
                        # would block the ACT sequencer mid-exp for ~667ns
                        oeng = nc.sync if hcn % 2 == 0 else nc.gpsimd
                        oeng.dma_start(
                            out_d[qt * 128:(qt + 1) * 128, hcn * 512:(hcn + 1) * 512],
                            ob[:])

        with ExitStack() as opj:
            obuf2 = opj.enter_context(tc.tile_pool(name="ob2", bufs=8))
            po_p = opj.enter_context(tc.tile_pool(name="po2", bufs=6, space="PSUM"))
            qc = QCN - 1
            for qt4 in range(4):
                for hcn in range(4):
                    po = po_p.tile([128, 512], f32, tag="po2")
                    for h in range(HPC):
                        nc.tensor.matmul(
                            po[:], ot_sb[h][qc][:, qt4 * 128:(qt4 + 1) * 128],
                            wo_sb[:, h, hcn * 512:(hcn + 1) * 512],
                            start=(h == 0), stop=(h == HPC - 1))
                    ob = obuf2.tile([128, 512], f32, tag="ob2")
                    if hcn % 2 == 0:
                        nc.scalar.copy(ob[:], po[:])
                    else:
                        nc.vector.tensor_copy(ob[:], po[:])
                    qt = qc * 4 + qt4
                    oeng = nc.sync if hcn % 2 == 0 else nc.gpsimd
                    oeng.dma_start(
                        out_d[qt * 128:(qt + 1) * 128, hcn * 512:(hcn + 1) * 512],
                        ob[:])

    nc.compile()
    return nc


def _prep_inputs(x, Wq, Wk, Wv, Wo):
    xT = np.ascontiguousarray(x.reshape(L, H).T).astype(np.float32)
    cosT, sinTs = _rope_tables()
    maskb = _mask_bias()
    ident = np.eye(128, dtype=np.float32)
    scale = np.float32(1.0 / np.sqrt(D))
    in_maps = []
    for i in range(NCORES):
        rs = slice(i * HPC * D, (i + 1) * HPC * D)
        in_maps.append({
            "xT": xT,
            "wqT": np.ascontiguousarray(Wq[rs].T * scale).reshape(16, 128, HPC * D),
            "wkT": np.ascontiguousarray(Wk[rs].T).reshape(16, 128, HPC * D),
            "wvT": np.ascontiguousarray(Wv[rs].T).reshape(16, 128, HPC * D),
            "woP": np.ascontiguousarray(Wo[:, rs].T),
            "cosT": cosT,
            "sinTs": sinTs,
            "maskb": maskb,
            "ident": ident,
        })
    return in_maps


def run(x, Wq, Wk, Wv, Wo, trace=False):
    from concourse.bass_utils import run_bass_kernel_spmd
    if "nc" not in _CACHE:
        _CACHE["nc"] = _build_nc()
    nc = _CACHE["nc"]
    in_maps = _prep_inputs(np.asarray(x), np.asarray(Wq), np.asarray(Wk),
                           np.asarray(Wv), np.asarray(Wo))
    res = run_bass_kernel_spmd(nc, in_maps, core_ids=list(range(NCORES)),
                               trace=trace)
    acc = np.zeros((L, H), dtype=np.float64)
    for r in res.results:
        acc += r["out"].astype(np.float64)
    return acc.astype(np.float32).reshape(1, L, H), res


def kernel(x, Wq, Wk, Wv, Wo):
    out, _ = run(x, Wq, Wk, Wv, Wo)
    return out


# revision 28
# speedup vs baseline: 1.0036x; 1.0036x over previous
"""Causal attention (RoPE, 16 heads, L=2048, H=2048) on 8 trn2 NeuronCores.

Sharding: tensor-parallel over heads. Core i handles heads 2i, 2i+1
(d=128 each): column-parallel q/k/v projections, row-parallel o_proj,
host-side sum of the 8 partial outputs.

Per-core device kernel (all matmuls fp32r = full-rate PE):
  - Q^T, K^T computed in [d, L] layout (weight-stationary matmuls, N=512),
    RoPE applied during PSUM->SBUF eviction on DVE.
  - V computed via PE transposes into natural [L, d] layout.
  - Weights streamed in kc-paired chunks on both hwdge queues, interleaved
    with x tiles (the lc=0 pass is DMA-bandwidth-bound).
  - Attention per (head, q-chunk of 512): S^T = K Q^T on PE; causal mask
    applied as an additive -1e30 bias via an identity matmul on the PE
    itself (no cross-engine dependency in the S->exp chain); diagonal
    blocks narrowed to the causal frontier; exp on ACT; softmax
    denominators via an all-ones matmul; unnormalized O^T accumulated
    over k-tiles; normalized by DVE reciprocal+mul into [d, L] layout.
  - o_proj: out_partial[q, H] from lhsT=O^T chunks, rhs=Wo slice.
"""
import numpy as np

L = 2048
H = 2048
NH = 16
D = 128          # head dim
NCORES = 8
HPC = NH // NCORES   # heads per core = 2
ROPE_BASE = 10000.0

_CACHE = {}


def _rope_tables():
    inv_freq = 1.0 / (ROPE_BASE ** (np.arange(0, D, 2, dtype=np.float32) / D))
    t = np.arange(L, dtype=np.float32)
    freqs = np.outer(t, inv_freq).astype(np.float32)          # [L, D/2]
    emb = np.concatenate([freqs, freqs], axis=-1)             # [L, D]
    cos = np.cos(emb).astype(np.float32)                      # [L, D]
    sin = np.sin(emb).astype(np.float32)
    cosT = np.ascontiguousarray(cos.T)                        # [D, L]
    sinT = np.ascontiguousarray(sin.T)
    sinTs = sinT.copy()
    sinTs[: D // 2] = -sinT[: D // 2]                         # sign-folded
    # partition-swapped so DVE operand base partitions match:
    # sinsw[p] = sinTs[(p+64) % 128]
    sinsw = np.concatenate([sinTs[D // 2:], sinTs[: D // 2]], axis=0)
    return cosT, np.ascontiguousarray(sinsw)


def _mask_bias():
    # maskb[j][k, q] = 0 where q - k - 128*j >= 0 else -1e30
    k = np.arange(128)[None, :, None]
    q = np.arange(512)[None, None, :]
    j = np.arange(4)[:, None, None]
    keep = (q - k - 128 * j) >= 0
    return np.where(keep, np.float32(0.0), np.float32(-1e30))


def _build_nc():
    import concourse.bacc as bacc
    import concourse.mybir as mybir
    from concourse import tile
    from contextlib import ExitStack

    f32 = mybir.dt.float32
    f32r = mybir.dt.float32r
    AF = mybir.ActivationFunctionType
    OP = mybir.AluOpType

    nc = bacc.Bacc("TRN2", target_bir_lowering=False, debug=False)

    xT_d = nc.dram_tensor("xT", (H, L), f32r, kind="ExternalInput")
    wq_d = nc.dram_tensor("wqT", (H // 128, 128, HPC * D), f32r, kind="ExternalInput")
    wk_d = nc.dram_tensor("wkT", (H // 128, 128, HPC * D), f32r, kind="ExternalInput")
    wv_d = nc.dram_tensor("wvT", (H // 128, 128, HPC * D), f32r, kind="ExternalInput")
    wo_d = nc.dram_tensor("woP", (HPC * D, H), f32r, kind="ExternalInput")
    cos_d = nc.dram_tensor("cosT", (D, L), f32, kind="ExternalInput")
    sin_d = nc.dram_tensor("sinTs", (D, L), f32, kind="ExternalInput")
    mb_d = nc.dram_tensor("maskb", (4, 128, 512), f32r, kind="ExternalInput")
    id_d = nc.dram_tensor("ident", (128, 128), f32r, kind="ExternalInput")
    out_d = nc.dram_tensor("out", (L, H), f32, kind="ExternalOutput")

    KC = H // 128        # 16 contraction chunks
    LCN = 4              # L chunks of 512 in projections
    QCN = 4              # q chunks of 512 in attention

    with tile.TileContext(nc) as tc, ExitStack() as top:
        per = top.enter_context(tc.tile_pool(name="per", bufs=1))

        wo_sb = per.tile([128, HPC, H], f32r)
        cos_sb = per.tile([128, L], f32)
        sin_sb = per.tile([128, L], f32)
        maskb_sb = per.tile([128, 4, 512], f32r)
        qt_sb = [[per.tile([128, 512], f32r, name=f"qt{h}_{c}")
                  for c in range(LCN)] for h in range(HPC)]
        kt_sb = [[per.tile([128, 512], f32r, name=f"kt{h}_{c}")
                  for c in range(LCN)] for h in range(HPC)]
        v_sb = [[per.tile([128, 4, D], f32r, name=f"v{h}_{c}")
                 for c in range(LCN)] for h in range(HPC)]
        ot_sb = [[per.tile([128, 512], f32r, name=f"ot{h}_{c}")
                  for c in range(QCN)] for h in range(HPC)]
        ones_f = per.tile([128, 128], f32)
        ones_r = per.tile([128, 128], f32r)
        ident = per.tile([128, 128], f32r)

        nc.vector.memset(ones_f[:], 1.0)
        nc.vector.tensor_copy(ones_r[:], ones_f[:])
        nc.gpsimd.dma_start(ident[:], id_d[:])

        # ---------------- projections ----------------
        with ExitStack() as proj:
            wpool = proj.enter_context(tc.tile_pool(name="w", bufs=1))
            xpool = proj.enter_context(tc.tile_pool(name="x", bufs=10))
            tpool = proj.enter_context(tc.tile_pool(name="ropetmp", bufs=3))
            spool = proj.enter_context(tc.tile_pool(name="stg", bufs=1))
            vtpool = proj.enter_context(tc.tile_pool(name="vt", bufs=3))
            qpps = proj.enter_context(tc.tile_pool(name="qpps", bufs=2, space="PSUM"))
            pps = proj.enter_context(tc.tile_pool(name="pps", bufs=1, space="PSUM"))

            wq_sb = wpool.tile([128, KC, HPC * D], f32r)
            wk_sb = wpool.tile([128, KC, HPC * D], f32r)
            wv_sb = wpool.tile([128, KC, HPC * D], f32r)
            wqr, wkr, wvr = wq_d, wk_d, wv_d

            # weight chunks: first kc alone (gates the first matmul), then
            # pairs; all three types split across the two hwdge queues
            # (gpsimd/swdge left completely unused to skip its drain)
            wchunks = [(0, 1), (1, 3), (3, 5), (5, 7), (7, 9), (9, 11),
                       (11, 13), (13, 16)]

            def wdma(eng, dst, src, ks):
                eng.dma_start(dst[:, ks, :], src[ks].rearrange("c p n -> p c n"))

            def rope(src, dst, cs):
                t1 = tpool.tile([128, 512], f32, tag="t1", name="t1")
                t2 = tpool.tile([128, 512], f32, tag="t2", name="t2")
                nc.vector.tensor_tensor(
                    t1[0:64, :], src[64:128, :], sin_sb[64:128, cs], OP.mult)
                nc.vector.tensor_tensor(
                    t1[64:128, :], src[0:64, :], sin_sb[0:64, cs], OP.mult)
                nc.vector.tensor_tensor(t2[:], src[:], cos_sb[:, cs], OP.mult)
                nc.vector.tensor_tensor(dst[:], t1[:], t2[:], OP.add)

            for lc in range(LCN):
                qps = [qpps.tile([128, 512], f32, tag=f"qps{h}", name=f"qps{h}") for h in range(HPC)]
                kps = [pps.tile([128, 512], f32, tag=f"kps{h}", name=f"kps{h}") for h in range(HPC)]
                vps = [pps.tile([128, 512], f32, tag=f"vps{i}", name=f"vps{i}") for i in range(HPC)]
                for kc in range(KC):
                    xt = xpool.tile([128, 512], f32r, tag="xt")
                    nc.sync.dma_start(
                        xt[:], xT_d[kc * 128:(kc + 1) * 128, lc * 512:(lc + 1) * 512])
                    if lc == 0:
                        for ci, (lo, hi) in enumerate(wchunks):
                            if lo == kc:
                                ks = slice(lo, hi)
                                wdma(nc.scalar, wq_sb, wqr, ks)
                                wdma(nc.sync, wk_sb, wkr, ks)
                                wdma(nc.gpsimd, wv_sb, wvr, ks)
                        if kc in (8, 12):
                            # cos/sin chunk c needed by RoPE of lc=c only;
                            # keep them out of the front DMA crunch
                            cs = slice((kc - 8) // 4 * 512, ((kc - 8) // 4 + 1) * 512)
                            nc.scalar.dma_start(cos_sb[:, cs], cos_d[:, cs])
                            nc.sync.dma_start(sin_sb[:, cs], sin_d[:, cs])
                    elif lc == 1:
                        if kc in (0, 4):
                            cs = slice((kc + 8) // 4 * 512, ((kc + 8) // 4 + 1) * 512)
                            nc.scalar.dma_start(cos_sb[:, cs], cos_d[:, cs])
                            nc.sync.dma_start(sin_sb[:, cs], sin_d[:, cs])
                        if kc == 8:
                            # mask biases aren't needed until attention
                            nc.gpsimd.dma_start(maskb_sb[:],
                                                mb_d.rearrange("j p n -> p j n"))
                    st, sp = (kc == 0), (kc == KC - 1)
                    for h in range(HPC):
                        nc.tensor.matmul(qps[h][:], wq_sb[:, kc, h * D:(h + 1) * D],
                                         xt[:], start=st, stop=sp)
                        nc.tensor.matmul(kps[h][:], wk_sb[:, kc, h * D:(h + 1) * D],
                                         xt[:], start=st, stop=sp)
                        nc.tensor.matmul(vps[h][:], wv_sb[:, kc, h * D:(h + 1) * D],
                                         xt[:], start=st, stop=sp)
                # evict psums to SBUF with the bank-gating copies first
                # (k/q banks gate the next chunk and the attention phase),
                # split across ACT and DVE so the drain halves; V^T staging
                # last (its banks aren't needed until o_proj).
                # RoPE: dst = src*cos + swap(src)*sin_signed  (DVE, f32r out)
                cs = slice(lc * 512, (lc + 1) * 512)
                stgk, stgq = [], []
                for h in range(HPC):
                    stg = spool.tile([128, 512], f32, tag=f"ks{h}", name=f"ks{h}")
                    if h == 0:
                        nc.scalar.copy(stg[:], kps[h][:])
                    else:
                        nc.vector.tensor_copy(stg[:], kps[h][:])
                    stgk.append(stg)
                for h in range(HPC):
                    stg = spool.tile([128, 512], f32, tag=f"qs{h}", name=f"qs{h}")
                    if h == 0:
                        nc.scalar.copy(stg[:], qps[h][:])
                    else:
                        nc.vector.tensor_copy(stg[:], qps[h][:])
                    stgq.append(stg)
                vts = []
                for h in range(HPC):
                    vt = vtpool.tile([128, 512], f32r, tag="vt")
                    nc.scalar.copy(vt[:], vps[h][:])
                    vts.append(vt)
                for h in range(HPC):
                    rope(stgq[h], qt_sb[h][lc], cs)
                    rope(stgk[h], kt_sb[h][lc], cs)
                # V^T -> natural V via PE transposes
                for h in range(HPC):
                    for lt in range(4):
                        tp = pps.tile([128, 128], f32r, tag=f"vps{h}", name="tp")
                        nc.tensor.transpose(tp[:], vts[h][:, lt * 128:(lt + 1) * 128],
                                            ident[:])
                        nc.scalar.copy(v_sb[h][lc][:, lt, :], tp[:])

        # -------- attention (qc-major) interleaved with o_proj --------
        with ExitStack() as att:
            ppool = att.enter_context(tc.tile_pool(name="pt", bufs=8))
            rpool = att.enter_context(tc.tile_pool(name="recip", bufs=2))
            obuf = att.enter_context(tc.tile_pool(name="ob", bufs=10))
            sps_p = att.enter_context(tc.tile_pool(name="sps", bufs=2, space="PSUM"))
            acc_p = att.enter_context(tc.tile_pool(name="acc", bufs=2, space="PSUM"))
            ops_p = att.enter_context(tc.tile_pool(name="ops", bufs=2, space="PSUM"))

            for qc in range(QCN):
                n_kt = 4 * qc + 4
                # interleave the two heads' chains so the PE always has an
                # independent stream while one head's exp is in flight
                sums = [acc_p.tile([128, 512], f32, tag="sums", name=f"sums{h}")
                        for h in range(HPC)]
                ops = [acc_p.tile([128, 512], f32, tag="ops", name=f"ops{h}")
                       for h in range(HPC)]
                for kt in range(n_kt):
                    st, sp = (kt == 0), (kt == n_kt - 1)
                    diag = kt >= 4 * qc
                    # diagonal blocks: columns below the causal frontier are
                    # fully masked -- skip them. j=3 keeps width 256 (a
                    # 128-wide fp32r matmul drops to 1/4 rate).
                    off = min(128 * (kt - 4 * qc), 256) if diag else 0
                    qs = slice(off, 512)
                    pts = []
                    for h in range(HPC):
                        s_ps = sps_p.tile([128, 512], f32, tag="s")
                        nc.tensor.matmul(
                            s_ps[:, qs], kt_sb[h][kt // 4][:, (kt % 4) * 128:(kt % 4 + 1) * 128],
                            qt_sb[h][qc][:, qs], start=True, stop=not diag)
                        if diag:
                            # causal mask: additive -1e30 bias injected by an
                            # identity matmul on the PE itself; only the
                            # 256-wide stripe around the frontier is nonzero
                            j = kt - 4 * qc
                            bs = slice(off, off + 256)
                            nc.tensor.matmul(s_ps[:, bs], ident[:],
                                             maskb_sb[:, j, bs],
                                             start=False, stop=True)
                        pt = ppool.tile([128, 512], f32r, tag="pt")
                        nc.scalar.activation(pt[:, qs], s_ps[:, qs], AF.Exp)
                        pts.append(pt)
                    # both heads' denominator matmuls back-to-back: they
                    # share the all-ones stationary operand
                    for h in range(HPC):
                        nc.tensor.matmul(sums[h][:, qs], ones_r[:], pts[h][:, qs],
                                         start=st, stop=sp)
                    for h in range(HPC):
                        nc.tensor.matmul(ops[h][:, qs], v_sb[h][kt // 4][:, kt % 4, :],
                                         pts[h][:, qs], start=st, stop=sp)
                if qc == 0:
                    # wo arrives during the attention phase, well before the
                    # first o_proj; issued on sync so the ACT sequencer (busy
                    # with the exp chain) never stalls on DMA setup
                    nc.sync.dma_start(wo_sb[:],
                                      wo_d.rearrange("(c p) n -> p c n", p=128))
                from contextlib import nullcontext
                prio = tc.high_priority() if qc == QCN - 1 else nullcontext()
                with prio:
                    # last chunk: hint the scheduler to slot the normalize
                    # ahead of pending o_proj evictions on the DVE queue
                    for h in range(HPC):
                        recip = rpool.tile([128, 512], f32, tag="recip")
                        nc.vector.reciprocal_approx_fast(recip[:], sums[h][:])
                        nc.vector.tensor_tensor(ot_sb[h][qc][:], ops[h][:],
                                                recip[:], OP.mult)
                # o_proj for the q-tiles of this q-chunk (overlaps the next
                # q-chunk's attention); the last chunk runs after the att
                # pools close, with a deeper dedicated pool
                if qc == QCN - 1:
                    continue
                for qt4 in range(4):
                    for hcn in range(4):
                        po = ops_p.tile([128, 512], f32, tag="po")
                        for h in range(HPC):
                            nc.tensor.matmul(
                                po[:], ot_sb[h][qc][:, qt4 * 128:(qt4 + 1) * 128],
                                wo_sb[:, h, hcn * 512:(hcn + 1) * 512],
                                start=(h == 0), stop=(h == HPC - 1))
                        ob = obuf.tile([128, 512], f32, tag="ob")
                        nc.vector.tensor_copy(ob[:], po[:])
                        qt = qc * 4 + qt4
                        # out-DMAs on sync/swdge only: a scalar-queue issue
                        # would block the ACT sequencer mid-exp for ~667ns
                        oeng = nc.sync if hcn % 2 == 0 else nc.gpsimd
                        oeng.dma_start(
                            out_d[qt * 128:(qt + 1) * 128, hcn * 512:(hcn + 1) * 512],
                            ob[:])

        with ExitStack() as opj:
            obuf2 = opj.enter_context(tc.tile_pool(name="ob2", bufs=8))
            po_p = opj.enter_context(tc.tile_pool(name="po2", bufs=6, space="PSUM"))
            qc = QCN - 1
            for qt4 in range(4):
                for hcn in range(4):
                    po = po_p.tile([128, 512], f32, tag="po2")
                    for h in range(HPC):
                        nc.tensor.matmul(
                            po[:], ot_sb[h][qc][:, qt4 * 128:(qt4 + 1) * 128],
                            wo_sb[:, h, hcn * 512:(hcn + 1) * 512],
                            start=(h == 0), stop=(h == HPC - 1))
                    ob = obuf2.tile([128, 512], f32, tag="ob2")
                    if hcn % 2 == 0:
                        nc.scalar.copy(ob[:], po[:])
                    else:
                        nc.vector.tensor_copy(ob[:], po[:])
                    qt = qc * 4 + qt4
                    oeng = nc.sync if hcn % 2 == 0 else nc.gpsimd
                    oeng.dma_start(
                        out_d[qt * 128:(qt + 1) * 128, hcn * 512:(hcn + 1) * 512],
                        ob[:])

    nc.compile()
    return nc


def _prep_inputs(x, Wq, Wk, Wv, Wo):
    xT = np.ascontiguousarray(x.reshape(L, H).T).astype(np.float32)
    cosT, sinTs = _rope_tables()
    maskb = _mask_bias()
    ident = np.eye(128, dtype=np.float32)
    scale = np.float32(1.0 / np.sqrt(D))
    in_maps = []
    for i in range(NCORES):
        rs = slice(i * HPC * D, (i + 1) * HPC * D)
        in_maps.append({
            "xT": xT,
            "wqT": np.ascontiguousarray(Wq[rs].T * scale).reshape(16, 128, HPC * D),
            "wkT": np.ascontiguousarray(Wk[rs].T).reshape(16, 128, HPC * D),
            "wvT": np.ascontiguousarray(Wv[rs].T).reshape(16, 128, HPC * D),
            "woP": np.ascontiguousarray(Wo[:, rs].T),
            "cosT": cosT,
            "sinTs": sinTs,
            "maskb": maskb,
            "ident": ident,
        })
    return in_maps


def run(x, Wq, Wk, Wv, Wo, trace=False):
    from concourse.bass_utils import run_bass_kernel_spmd
    if "nc" not in _CACHE:
        _CACHE["nc"] = _build_nc()
    nc = _CACHE["nc"]
    in_maps = _prep_inputs(np.asarray(x), np.asarray(Wq), np.asarray(Wk),
                           np.asarray(Wv), np.asarray(Wo))
    res = run_bass_kernel_spmd(nc, in_maps, core_ids=list(range(NCORES)),
                               trace=trace)
    acc = np.zeros((L, H), dtype=np.float64)
    for r in res.results:
        acc += r["out"].astype(np.float64)
    return acc.astype(np.float32).reshape(1, L, H), res


def kernel(x, Wq, Wk, Wv, Wo):
    out, _ = run(x, Wq, Wk, Wv, Wo)
    return out
